# revision 1
# baseline (speedup 1.0000x reference)
"""GAT (nn_GAT_29523605193094) Trainium2 kernel.

The reference keeps the source bug ``src, dst = edges[0], edges[0]``, so the
adjacency matrix is purely diagonal: adj[i, i] = (i appears in edges[0]).
After the -inf masking, row i of the [N, N, H] score tensor has exactly one
finite entry (j = i) when node i is covered, so softmax over axis=1 yields
exactly 1.0 at (i, i) and 0.0 elsewhere, and the output row is exactly
h[i] = (X @ W)[i].  Rows for uncovered nodes are all -inf -> softmax is NaN
-> the output row is NaN.  Both cases are reproduced here bit-exactly:

    out = X @ W            (on 8 NeuronCores, row-sharded)
    out[~covered] = NaN    (host-side mask from edges[0])

The device work is a row-sharded [4096, 512] @ [512, 256] fp32 matmul.
Each core gets 512 rows of X (pre-transposed on host to the [K, M] layout
the PE wants for the stationary operand) plus the full W.
"""

import numpy as np

N = 4096
IN = 512
OUT = 256
NCORES = 8
RB = N // NCORES  # 512 rows per core
P = 128
KT = IN // P      # 4 contraction chunks
MT = RB // P      # 4 output row blocks per core

_state = {}

# test.py reads this after a traced call for the HW exec time.
LAST_RESULTS = None


def _build():
    import concourse.mybir as mybir
    import concourse.tile as tile
    from concourse import bacc
    from concourse.bass import ts

    nc = bacc.Bacc(
        "TRN2",
        target_bir_lowering=False,
        debug=False,
        num_devices=NCORES,
    )
    f32 = mybir.dt.float32
    xt = nc.dram_tensor("xt", [IN, RB], f32, kind="ExternalInput")  # X_shard^T
    w = nc.dram_tensor("w", [IN, OUT], f32, kind="ExternalInput")
    out = nc.dram_tensor("out", [RB, OUT], f32, kind="ExternalOutput")

    with tile.TileContext(nc) as tc:
        with (
            tc.tile_pool(name="ins", bufs=1) as in_pool,
            tc.tile_pool(name="outs", bufs=4) as out_pool,
            tc.tile_pool(name="ps", bufs=4, space="PSUM") as psum_pool,
        ):
            xt_t = in_pool.tile([P, KT, RB], f32)
            w_t = in_pool.tile([P, KT, OUT], f32)
            # Two HWDGE queues (sync for xt, scalar for w) so the transfers
            # pipeline in parallel; chunks are ordered the way the matmul
            # loop consumes them.  xt chunk 0 is split at column 256: the
            # first half feeds the (m0, m1) phase immediately, the second
            # half (m2, m3 slices) is only needed once that phase is done.
            HB = 2 * P  # 256: column split point of xt chunk 0
            nc.sync.dma_start(xt_t[:, 0, 0:HB], xt[ts(0, P), 0:HB])
            for k in range(1, KT):
                nc.sync.dma_start(xt_t[:, k, :], xt[ts(k, P), :])
            nc.sync.dma_start(xt_t[:, 0, HB:RB], xt[ts(0, P), HB:RB])
            for k in range(KT):
                nc.scalar.dma_start(w_t[:, k, :], w[ts(k, P), :])

            # Process m-blocks in pairs with k inner so each arriving
            # (xt_k, w_k) chunk pair feeds ~0.9us of PE work (two m-blocks)
            # instead of ~0.43us — the PE stays saturated during the input
            # stream, and the first pair's output DMAs + HBM write receipts
            # retire while the second pair is still computing.
            for pair in range(MT // 2):
                pss = [
                    psum_pool.tile([P, OUT], f32, name=f"ps{pair}_{i}", tag="ps")
                    for i in range(2)
                ]
                for k in range(KT):
                    for i in range(2):
                        m = 2 * pair + i
                        nc.tensor.matmul(
                            pss[i][:],
                            xt_t[:, k, ts(m, P)],
                            w_t[:, k, :],
                            start=(k == 0),
                            stop=(k == KT - 1),
                        )
                for i in range(2):
                    m = 2 * pair + i
                    ob = out_pool.tile([P, OUT], f32)
                    nc.vector.tensor_copy(ob[:], pss[i][:])
                    # Alternate output queues so the HBM write receipts
                    # overlap instead of serializing.
                    (nc.scalar if i == 0 else nc.sync).dma_start(
                        out[ts(m, P), :], ob[:]
                    )

    nc.compile()
    return nc


def kernel(X, edges, W, A):
    global LAST_RESULTS
    from concourse.bass_utils import run_bass_kernel_spmd

    X = np.ascontiguousarray(np.asarray(X, dtype=np.float32))
    W = np.ascontiguousarray(np.asarray(W, dtype=np.float32))
    edges = np.asarray(edges)

    if "nc" not in _state:
        _state["nc"] = _build()
    nc = _state["nc"]

    XT = np.ascontiguousarray(X.T)  # [IN, N]
    in_maps = [
        {"xt": np.ascontiguousarray(XT[:, c * RB : (c + 1) * RB]), "w": W}
        for c in range(NCORES)
    ]
    # The device occasionally reports a transient NRT_EXEC_UNIT_UNRECOVERABLE
    # on an otherwise-good kernel; retry before giving up.
    last_exc = None
    for _attempt in range(3):
        try:
            res = run_bass_kernel_spmd(nc, in_maps, core_ids=list(range(NCORES)))
            break
        except Exception as exc:  # noqa: BLE001
            last_exc = exc
            import time

            time.sleep(2.0)
    else:
        raise last_exc
    LAST_RESULTS = res
    out = np.concatenate([res.results[c]["out"] for c in range(NCORES)], axis=0)

    # Reference semantics: nodes absent from edges[0] have an all -inf score
    # row; softmax of that is NaN, which propagates to the output row.
    covered = np.zeros(N, dtype=bool)
    covered[edges[0]] = True
    if not covered.all():
        out[~covered] = np.nan
    return out



# revision 3
# speedup vs baseline: 1.1395x; 1.1395x over previous
"""GAT (nn_GAT_29523605193094) Trainium2 kernel.

The reference keeps the source bug ``src, dst = edges[0], edges[0]``, so the
adjacency matrix is purely diagonal: adj[i, i] = (i appears in edges[0]).
After the -inf masking, row i of the [N, N, H] score tensor has exactly one
finite entry (j = i) when node i is covered, so softmax over axis=1 yields
exactly 1.0 at (i, i) and 0.0 elsewhere, and the output row is exactly
h[i] = (X @ W)[i].  Rows for uncovered nodes are all -inf -> softmax is NaN
-> the output row is NaN.  Both cases are reproduced here:

    out = X @ W            (on 8 NeuronCores, row-sharded)
    out[~covered] = NaN    (host-side mask from edges[0])

The device work is a row-sharded [4096, 512] @ [512, 256] matmul.  Each core
gets 512 rows of X.  Inputs are marshalled to bf16 on the host (tolerance is
2e-2; bf16 with fp32 PSUM accumulation lands ~2e-3), which both halves the
HBM traffic and lets the PE run single-pass (fp32 needs the 2x LOW_HIGH
replay).  Layouts are packed so every DMA descriptor covers a full 2-4 KiB
per-partition row, and each transfer is split across both HWDGE queues
(sync + scalar) by partition halves so all 16 DMA engines stream at once.
A short burst of dummy matmuls warms the PE clock out of its low p-state
while the input DMAs are in flight.
"""

import numpy as np

N = 4096
IN = 512
OUT = 256
NCORES = 8
RB = N // NCORES  # 512 rows per core
P = 128
KT = IN // P      # 4 contraction chunks
MT = RB // P      # 4 output row blocks per core
HP = P // 2       # partition half for dual-queue DMA splits
KW = KT * RB      # xb free width (bf16 cols)
WW = KT * OUT     # wb free width
OW = MT * OUT     # outb free width
N_WARM = 8        # PE p-state warmup matmuls

_state = {}

# test.py reads this after a traced call for the HW exec time.
LAST_RESULTS = None


def _build():
    import concourse.mybir as mybir
    import concourse.tile as tile
    from concourse import bacc
    from concourse.bass import ts

    nc = bacc.Bacc(
        "TRN2",
        target_bir_lowering=False,
        debug=False,
        num_devices=NCORES,
    )
    f32 = mybir.dt.float32
    bf16 = mybir.dt.bfloat16
    # xb[p, k*RB + c] = X[core*RB + c, k*128 + p]   (bf16, 4 KiB rows)
    # wb[p, k*OUT + f] = W[k*128 + p, f]            (bf16, 2 KiB rows)
    # outb[p, m*OUT + f] = (X @ W)[core*RB + m*128 + p, f]  (f32, 4 KiB rows)
    xb = nc.dram_tensor("xb", [P, KW], bf16, kind="ExternalInput")
    wb = nc.dram_tensor("wb", [P, WW], bf16, kind="ExternalInput")
    outb = nc.dram_tensor("outb", [P, OW], f32, kind="ExternalOutput")

    with tile.TileContext(nc) as tc:
        with (
            tc.tile_pool(name="ins", bufs=1) as in_pool,
            tc.tile_pool(name="warm", bufs=1) as warm_pool,
            tc.tile_pool(name="outs", bufs=2) as out_pool,
            tc.tile_pool(name="ps", bufs=4, space="PSUM") as psum_pool,
            tc.tile_pool(name="psw", bufs=1, space="PSUM") as psw_pool,
        ):
            xb_t = in_pool.tile([P, KW], bf16)
            wb_t = in_pool.tile([P, WW], bf16)
            # Split every transfer across both HWDGE queues by partition
            # halves: each DMA is 64 descriptors of a full per-partition row,
            # so the two queues together keep all 16 DMA engines streaming.
            nc.sync.dma_start(xb_t[0:HP, :], xb[0:HP, :])
            nc.scalar.dma_start(xb_t[HP:P, :], xb[HP:P, :])
            nc.sync.dma_start(wb_t[0:HP, :], wb[0:HP, :])
            nc.scalar.dma_start(wb_t[HP:P, :], wb[HP:P, :])

            # Warm the PE out of its low p-state while the inputs stream in:
            # dummy matmuls on a zeroed tile into a scratch PSUM bank.  The
            # clock ramps with continuous busy time, so the real matmuls
            # below start at speed instead of at 0.65 GHz.
            warm_t = warm_pool.tile([P, P + OUT], bf16)
            nc.gpsimd.memset(warm_t[:], 0.0)
            ps_w = psw_pool.tile([P, OUT], f32, name="psw", tag="psw")
            for _ in range(N_WARM):
                nc.tensor.matmul(
                    ps_w[:], warm_t[:, 0:P], warm_t[:, P : P + OUT],
                    start=True, stop=True,
                )

            # m-outer / k-inner: each m-block's PSUM completes after its 4
            # accumulating matmuls, so its copy + output DMA drain while the
            # next block computes.  bf16 operands stream 1 column/cycle.
            obs = [
                out_pool.tile([P, 2 * OUT], f32, name=f"ob{i}") for i in range(2)
            ]
            for m in range(MT):
                ps = psum_pool.tile([P, OUT], f32, name=f"ps{m}", tag="ps")
                for k in range(KT):
                    nc.tensor.matmul(
                        ps[:],
                        xb_t[:, k * RB + m * P : k * RB + (m + 1) * P],
                        wb_t[:, ts(k, OUT)],
                        start=(k == 0),
                        stop=(k == KT - 1),
                    )
                pair, half = divmod(m, 2)
                nc.vector.tensor_copy(obs[pair][:, ts(half, OUT)], ps[:])
                if half == 1:
                    # 2 KiB rows per descriptor; halves on separate queues.
                    nc.sync.dma_start(
                        outb[0:HP, ts(pair, 2 * OUT)], obs[pair][0:HP, :]
                    )
                    nc.scalar.dma_start(
                        outb[HP:P, ts(pair, 2 * OUT)], obs[pair][HP:P, :]
                    )

    nc.compile()
    return nc


def kernel(X, edges, W, A):
    global LAST_RESULTS
    import ml_dtypes
    from concourse.bass_utils import run_bass_kernel_spmd

    X = np.asarray(X, dtype=np.float32)
    W = np.asarray(W, dtype=np.float32)
    edges = np.asarray(edges)

    if "nc" not in _state:
        _state["nc"] = _build()
    nc = _state["nc"]

    bf16 = ml_dtypes.bfloat16
    XT = X.T  # [IN, N]
    wb_np = np.ascontiguousarray(
        W.reshape(KT, P, OUT).transpose(1, 0, 2).reshape(P, WW)
    ).astype(bf16)
    in_maps = []
    for c in range(NCORES):
        shard = XT[:, c * RB : (c + 1) * RB]  # [IN, RB]
        xb_np = np.ascontiguousarray(
            shard.reshape(KT, P, RB).transpose(1, 0, 2).reshape(P, KW)
        ).astype(bf16)
        in_maps.append({"xb": xb_np, "wb": wb_np})

    # The device occasionally reports a transient NRT_EXEC_UNIT_UNRECOVERABLE
    # on an otherwise-good kernel; retry before giving up.
    last_exc = None
    for _attempt in range(3):
        try:
            res = run_bass_kernel_spmd(nc, in_maps, core_ids=list(range(NCORES)))
            break
        except Exception as exc:  # noqa: BLE001
            last_exc = exc
            import time

            time.sleep(2.0)
    else:
        raise last_exc
    LAST_RESULTS = res
    out = np.concatenate(
        [
            np.asarray(res.results[c]["outb"], dtype=np.float32)
            .reshape(P, MT, OUT)
            .transpose(1, 0, 2)
            .reshape(RB, OUT)
            for c in range(NCORES)
        ],
        axis=0,
    )

    # Reference semantics: nodes absent from edges[0] have an all -inf score
    # row; softmax of that is NaN, which propagates to the output row.
    covered = np.zeros(N, dtype=bool)
    covered[edges[0]] = True
    if not covered.all():
        out[~covered] = np.nan
    return out
